# revision 24
# baseline (speedup 1.0000x reference)
"""Bidirectional LSTM layer on 8 trn2 NeuronCores.

Sharding: cores 0-3 forward direction x batch slices of 16,
cores 4-7 backward direction (input time-reversed on host) x batch slices.
Each core: input projection as windowed bulk GEMM + sequential recurrence.

All on-chip state transposed: h,c stored [H, b] (H on partitions).
Gates computed as gates^T = W^T-chunks . h  ->  PSUM [128, 16*b] per step,
column blocks ordered [g|o|i|f] so tanh(g)/sig(o) start before i/f matmuls
finish.
"""

import os

import numpy as np
import ml_dtypes

T, B, I, H = 512, 64, 512, 512
G4 = 4 * H            # 2048 gate rows
NCORES = 8
bp = B // 4           # 16 batch per core
W = 32                # steps per window
NWIN = T // W         # 16
NM = G4 // 128        # 16 m-chunks
NK = H // 128         # 4 k-chunks
GCOLS = NM * bp       # 256 gate cols per step
HC = NK * bp          # 64 cols for h/c tiles

USE_BF16 = True      # matmul dtype switch

# column block position for m-chunk within the step's gate tile.
# order [g(0-3) | o(4-7) | i(8-11) | f(12-15)]
def _pos_of_m(m):
    j = m % 4
    if m < 4:
        return 8 + j      # i
    if m < 8:
        return 12 + j     # f
    if m < 12:
        return 0 + j      # g
    return 4 + j          # o


# PE emit order for gate m-chunks: f, g, o, i — the trailing group (i)
# has the shortest dependent chain left after the gate burst ends.
_M_ORDER = [4, 5, 6, 7, 8, 9, 10, 11, 12, 13, 14, 15, 0, 1, 2, 3]


def build_nc(use_bf16):
    import concourse.bacc as bacc
    import concourse.mybir as mybir
    import concourse.tile as tile
    from concourse.tile_rust import add_dep_helper

    dtm = mybir.dt.bfloat16 if use_bf16 else mybir.dt.float32
    f32 = mybir.dt.float32
    Sig = mybir.ActivationFunctionType.Sigmoid
    Tanh = mybir.ActivationFunctionType.Tanh

    nc = bacc.Bacc("TRN2", target_bir_lowering=False, debug=False)

    xt = nc.dram_tensor("xt", [I, T * bp], dtm, kind="ExternalInput")
    wih = nc.dram_tensor("wih", [I, G4], dtm, kind="ExternalInput")
    whh = nc.dram_tensor("whh", [H, G4], dtm, kind="ExternalInput")
    bias = nc.dram_tensor("bias", [128, NM], f32, kind="ExternalInput")
    ident = nc.dram_tensor("ident", [128, 128], dtm, kind="ExternalInput")
    h0 = nc.dram_tensor("h0", [H, bp], dtm, kind="ExternalInput")
    c0 = nc.dram_tensor("c0", [H, bp], f32, kind="ExternalInput")
    out_h = nc.dram_tensor("out_h", [H, T * bp], f32, kind="ExternalOutput")
    h_fin = nc.dram_tensor("h_fin", [H, bp], f32, kind="ExternalOutput")
    c_fin = nc.dram_tensor("c_fin", [H, bp], f32, kind="ExternalOutput")

    with tile.TileContext(nc) as tc:
        with (
            tc.tile_pool(name="consts", bufs=1) as cpool,
            tc.tile_pool(name="xt", bufs=2) as xtpool,
            tc.tile_pool(name="xproj", bufs=2) as xppool,
            tc.tile_pool(name="outw", bufs=2) as outpool,
            tc.tile_pool(name="state", bufs=1) as spool,
            tc.tile_pool(name="hmm", bufs=3) as hpool,
            tc.tile_pool(name="chain", bufs=3) as chpool,
            tc.tile_pool(name="pp", bufs=2, space="PSUM") as pppool,
            tc.tile_pool(name="gp", bufs=2, space="PSUM") as gppool,
        ):
            whh_sb, wih_sb = [], []
            for k in range(NK):
                t = cpool.tile([128, G4], dtm, tag=f"whh{k}")
                nc.sync.dma_start(out=t[:, :], in_=whh[k * 128:(k + 1) * 128, :])
                whh_sb.append(t)
            for k in range(NK):
                t = cpool.tile([128, G4], dtm, tag=f"wih{k}")
                nc.sync.dma_start(out=t[:, :], in_=wih[k * 128:(k + 1) * 128, :])
                wih_sb.append(t)
            bias_sb = cpool.tile([128, NM], f32, tag="bias")
            nc.sync.dma_start(out=bias_sb[:, :], in_=bias[:, :])
            id_sb = cpool.tile([128, 128], dtm, tag="ident")
            nc.sync.dma_start(out=id_sb[:, :], in_=ident[:, :])

            # TC = [tanh_g | c] adjacent so fc/ig fuse into one DVE mul
            TC = spool.tile([128, 2 * HC], f32, tag="TC")
            for k in range(NK):
                nc.sync.dma_start(out=TC[:, HC + k * bp:HC + (k + 1) * bp],
                                  in_=c0[k * 128:(k + 1) * 128, :])
            hA = hpool.tile([128, 2 * bp], dtm, tag="hA")
            hB = hpool.tile([128, 2 * bp], dtm, tag="hB")
            for k in range(NK):
                dst = hA if k < 2 else hB
                nc.sync.dma_start(out=dst[:, (k % 2) * bp:(k % 2 + 1) * bp],
                                  in_=h0[k * 128:(k + 1) * 128, :])

            def emit_xt_dma(w):
                t = xtpool.tile([128, NK * W * bp], dtm, tag="xt")
                for k in range(NK):
                    nc.sync.dma_start(
                        out=t[:, k * W * bp:(k + 1) * W * bp],
                        in_=xt[k * 128:(k + 1) * 128, w * W * bp:(w + 1) * W * bp])
                return t

            def emit_proj_mm(t_xt, m):
                pp = pppool.tile([128, W * bp], f32, tag="pp")
                for k in range(NK):
                    nc.tensor.matmul(
                        pp[:, :],
                        lhsT=wih_sb[k][:, m * 128:(m + 1) * 128],
                        rhs=t_xt[:, k * W * bp:(k + 1) * W * bp],
                        start=(k == 0), stop=(k == NK - 1))
                return pp

            def emit_evac(xpw, pp, m):
                p = _pos_of_m(m)
                xpw3 = xpw[:, :].rearrange("p (w c) -> p w c", c=GCOLS)
                dst = xpw3[:, :, p * bp:(p + 1) * bp]
                pp3 = pp[:, :].rearrange("p (w c) -> p w c", c=bp)
                return nc.vector.tensor_scalar_add(dst, pp3,
                                                   bias_sb[:, m:m + 1])

            def emit_gates(s, xpw, h_prev):
                hA, hB = h_prev
                gp_g = gppool.tile([128, HC], f32, tag="gpg")
                gp_oi = gppool.tile([128, 2 * HC], f32, tag="gpoi")
                gp_f = gppool.tile([128, HC], f32, tag="gpf")
                nc.tensor.matmul(gp_g[:, :], lhsT=id_sb[:, :],
                                 rhs=xpw[:, s * GCOLS:s * GCOLS + HC],
                                 start=True, stop=False, skip_group_check=True)
                nc.tensor.matmul(gp_oi[:, :], lhsT=id_sb[:, :],
                                 rhs=xpw[:, s * GCOLS + HC:s * GCOLS + 3 * HC],
                                 start=True, stop=False, skip_group_check=True)
                nc.tensor.matmul(gp_f[:, :], lhsT=id_sb[:, :],
                                 rhs=xpw[:, s * GCOLS + 3 * HC:(s + 1) * GCOLS],
                                 start=True, stop=False, skip_group_check=True)
                # soi/P allocated here so mid-gate chain ops can interleave:
                # tanh_g after g-groups, sig_oi after i-groups, P_ig right
                # after — all hidden under remaining gate matmuls.
                soi = chpool.tile([128, 2 * HC], f32, tag="soi")
                sf = chpool.tile([128, HC], f32, tag="sf")
                P = chpool.tile([128, 2 * HC], f32, tag="P")
                for mi, m in enumerate(_M_ORDER):
                    p = _pos_of_m(m)
                    if p < 4:
                        tgt, pc = gp_g, p
                    elif p < 12:
                        tgt, pc = gp_oi, p - 4
                    else:
                        tgt, pc = gp_f, p - 12
                    for k in range(NK):
                        hk = hA if k < 2 else hB
                        nc.tensor.matmul(
                            tgt[:, pc * bp:(pc + 1) * bp],
                            lhsT=whh_sb[k][:, m * 128:(m + 1) * 128],
                            rhs=hk[:, (k % 2) * bp:(k % 2 + 1) * bp],
                            start=False,
                            stop=(mi == NM - 1 and k == NK - 1),
                            skip_group_check=True)
                    if mi == 3:     # f-groups done: sig_f + f*c hide here
                        nc.scalar.activation(sf[:, :], gp_f[:, :], Sig)
                        nc.vector.tensor_mul(P[:, HC:2 * HC], sf[:, :],
                                             TC[:, HC:2 * HC])
                    elif mi == 7:   # g-groups done
                        nc.scalar.activation(TC[:, 0:HC], gp_g[:, :], Tanh)
                return gp_oi, soi, sf, P

            def emit_chain(gp_oi, soi, sf, P, h_new, hout_dst):
                hA, hB = h_new
                nc.scalar.activation(soi[:, :], gp_oi[:, :], Sig)
                nc.vector.tensor_mul(P[:, 0:HC], soi[:, HC:2 * HC],
                                     TC[:, 0:HC])
                nc.vector.tensor_add(TC[:, HC:2 * HC], P[:, 0:HC],
                                     P[:, HC:2 * HC])
                tcn = chpool.tile([128, HC], f32, tag="tcn")
                nc.scalar.activation(tcn[:, :], TC[:, HC:2 * HC], Tanh)
                mul1 = nc.vector.tensor_mul(hA[:, :], soi[:, 0:2 * bp],
                                            tcn[:, 0:2 * bp])
                mul2 = nc.vector.tensor_mul(hB[:, :], soi[:, 2 * bp:HC],
                                            tcn[:, 2 * bp:])
                nc.gpsimd.tensor_mul(hout_dst, soi[:, 0:HC], tcn[:, :])
                return mul2

            # prologue: window 0 projection
            t_xt = emit_xt_dma(0)
            xpw = xppool.tile([128, W * GCOLS], dtm, tag="xpw")
            for m in range(NM):
                pp0 = emit_proj_mm(t_xt, m)
                emit_evac(xpw, pp0, m)

            h_prev = (hA, hB)
            outw = None
            prev_tail = None      # last chain DVE op of previous step
            pending_evac = None   # (pp, m) awaiting evacuation next step
            for w in range(NWIN):
                xpw_next = None
                if w + 1 < NWIN:
                    t_xt_next = emit_xt_dma(w + 1)
                    xpw_next = xppool.tile([128, W * GCOLS], dtm, tag="xpw")
                outw = outpool.tile([128, W * HC], f32, tag="outw")
                for s in range(W):
                    gp_oi, soi, sf, P = emit_gates(s, xpw, h_prev)
                    if pending_evac is not None:
                        pp_p, m_p, xpw_p = pending_evac
                        with tc.high_priority(offset=-1000000):
                            ev = emit_evac(xpw_p, pp_p, m_p)
                        if prev_tail is not None:
                            add_dep_helper(ev.ins, prev_tail.ins, sync=False,
                                           reason="evac after prior chain")
                        pending_evac = None
                    if xpw_next is not None and s % 2 == 0:
                        with tc.high_priority(offset=-1000000):
                            pp = emit_proj_mm(t_xt_next, s // 2)
                        pending_evac = (pp, s // 2, xpw_next)
                    hA_n = hpool.tile([128, 2 * bp], dtm, tag="hA", name="hA_n")
                    hB_n = hpool.tile([128, 2 * bp], dtm, tag="hB", name="hB_n")
                    h_new = (hA_n, hB_n)
                    prev_tail = emit_chain(gp_oi, soi, sf, P, h_new,
                                           outw[:, s * HC:(s + 1) * HC])
                    h_prev = h_new
                ow3 = outw[:, :].rearrange("p (w c) -> p w c", c=HC)
                for k in range(NK):
                    nc.sync.dma_start(
                        out=out_h[k * 128:(k + 1) * 128,
                                  w * W * bp:(w + 1) * W * bp],
                        in_=ow3[:, :, k * bp:(k + 1) * bp])
                xpw = xpw_next

            for k in range(NK):
                nc.sync.dma_start(
                    out=h_fin[k * 128:(k + 1) * 128, :],
                    in_=outw[:, (W - 1) * HC + k * bp:(W - 1) * HC + (k + 1) * bp])
                nc.sync.dma_start(out=c_fin[k * 128:(k + 1) * 128, :],
                                  in_=TC[:, HC + k * bp:HC + (k + 1) * bp])

    nc.finalize()
    return nc


_NC_CACHE = {}


def _get_nc(use_bf16):
    if use_bf16 not in _NC_CACHE:
        _NC_CACHE[use_bf16] = build_nc(use_bf16)
    return _NC_CACHE[use_bf16]


def _mm_np_dtype(use_bf16):
    return ml_dtypes.bfloat16 if use_bf16 else np.float32


def kernel(x, h0_f, c0_f, h0_b, c0_b, w_ih_f, w_hh_f, b_ih_f, b_hh_f,
           w_ih_b, w_hh_b, b_ih_b, b_hh_b):
    from concourse.bass_utils import run_bass_kernel_spmd

    use_bf16 = USE_BF16
    dt = _mm_np_dtype(use_bf16)
    x = np.asarray(x, np.float32)
    fwd = (np.asarray(h0_f, np.float32), np.asarray(c0_f, np.float32),
           np.asarray(w_ih_f, np.float32), np.asarray(w_hh_f, np.float32),
           np.asarray(b_ih_f, np.float32), np.asarray(b_hh_f, np.float32))
    bwd = (np.asarray(h0_b, np.float32), np.asarray(c0_b, np.float32),
           np.asarray(w_ih_b, np.float32), np.asarray(w_hh_b, np.float32),
           np.asarray(b_ih_b, np.float32), np.asarray(b_hh_b, np.float32))

    nc = _get_nc(use_bf16)
    ident = np.eye(128, dtype=dt)
    in_maps = []
    for core in range(NCORES):
        d, si = divmod(core, 4)
        sl = slice(si * bp, (si + 1) * bp)
        h0_, c0_, wih_, whh_, bih_, bhh_ = fwd if d == 0 else bwd
        xd = x if d == 0 else np.ascontiguousarray(x[::-1])
        xslc = np.ascontiguousarray(
            xd[:, sl, :].transpose(2, 0, 1)).reshape(I, T * bp)
        in_maps.append({
            "xt": xslc.astype(dt),
            "wih": np.ascontiguousarray(wih_.T).astype(dt),
            "whh": np.ascontiguousarray(whh_.T).astype(dt),
            "bias": np.ascontiguousarray(
                (bih_ + bhh_).reshape(NM, 128).T).astype(np.float32),
            "ident": ident,
            "h0": np.ascontiguousarray(h0_[sl].T).astype(dt),
            "c0": np.ascontiguousarray(c0_[sl].T).astype(np.float32),
        })

    trace = os.environ.get("LSTM_TRACE", "0") == "1"
    kwargs = {}
    if trace:
        kwargs["trace"] = True
        td = os.environ.get("LSTM_TRACE_DIR")
        if td:
            kwargs["tmpdir"] = td
    res = run_bass_kernel_spmd(nc, in_maps, core_ids=list(range(NCORES)),
                               **kwargs)
    kernel._last_results = res

    outputs = np.empty((T, B, 2 * H), np.float32)
    hf = np.empty((B, H), np.float32)
    cf = np.empty((B, H), np.float32)
    hb = np.empty((B, H), np.float32)
    cb = np.empty((B, H), np.float32)
    for core in range(NCORES):
        d, si = divmod(core, 4)
        sl = slice(si * bp, (si + 1) * bp)
        r = res.results[core]
        seq = r["out_h"].reshape(H, T, bp).transpose(1, 2, 0)
        if d == 1:
            seq = seq[::-1]
        outputs[:, sl, d * H:(d + 1) * H] = seq
        (hf if d == 0 else hb)[sl] = r["h_fin"].T
        (cf if d == 0 else cb)[sl] = r["c_fin"].T
    return outputs, hf, cf, hb, cb


# revision 27
# speedup vs baseline: 1.1348x; 1.1348x over previous
"""Bidirectional LSTM layer on 8 trn2 NeuronCores.

Sharding: cores 0-3 forward direction x batch slices of 16,
cores 4-7 backward direction (input time-reversed on host) x batch slices.
Each core: input projection as windowed bulk GEMM + sequential recurrence.

All on-chip state transposed: h,c stored [H, b] (H on partitions).
Gates computed as gates^T = W^T-chunks . h  ->  PSUM [128, 16*b] per step,
column blocks ordered [g|o|i|f] so tanh(g)/sig(o) start before i/f matmuls
finish.
"""

import os

import numpy as np
import ml_dtypes

T, B, I, H = 512, 64, 512, 512
G4 = 4 * H            # 2048 gate rows
NCORES = 8
bp = B // 4           # 16 batch per core
W = 32                # steps per window
NWIN = T // W         # 16
NM = G4 // 128        # 16 m-chunks
NK = H // 128         # 4 k-chunks
GCOLS = NM * bp       # 256 gate cols per step
HC = NK * bp          # 64 cols for h/c tiles

USE_BF16 = True      # matmul dtype switch

# column block position for m-chunk within the step's gate tile.
# order [g(0-3) | o(4-7) | i(8-11) | f(12-15)]
def _pos_of_m(m):
    j = m % 4
    if m < 4:
        return 8 + j      # i
    if m < 8:
        return 12 + j     # f
    if m < 12:
        return 0 + j      # g
    return 4 + j          # o


# PE emit order for gate m-chunks: f, g, i, o — f/g/i consumers hide
# under later gate matmuls; only sig_o + the c/h tail trail the burst.
_M_ORDER = [4, 5, 6, 7, 8, 9, 10, 11, 0, 1, 2, 3, 12, 13, 14, 15]


def build_nc(use_bf16):
    import concourse.bacc as bacc
    import concourse.mybir as mybir
    import concourse.tile as tile
    from concourse.tile_rust import add_dep_helper

    dtm = mybir.dt.bfloat16 if use_bf16 else mybir.dt.float32
    f32 = mybir.dt.float32
    Sig = mybir.ActivationFunctionType.Sigmoid
    Tanh = mybir.ActivationFunctionType.Tanh

    nc = bacc.Bacc("TRN2", target_bir_lowering=False, debug=False)

    xt = nc.dram_tensor("xt", [I, T * bp], dtm, kind="ExternalInput")
    wih = nc.dram_tensor("wih", [I, G4], dtm, kind="ExternalInput")
    whh = nc.dram_tensor("whh", [H, G4], dtm, kind="ExternalInput")
    bias = nc.dram_tensor("bias", [128, NM], f32, kind="ExternalInput")
    ident = nc.dram_tensor("ident", [128, 128], dtm, kind="ExternalInput")
    h0 = nc.dram_tensor("h0", [H, bp], dtm, kind="ExternalInput")
    c0 = nc.dram_tensor("c0", [H, bp], f32, kind="ExternalInput")
    out_h = nc.dram_tensor("out_h", [H, T * bp], f32, kind="ExternalOutput")
    h_fin = nc.dram_tensor("h_fin", [H, bp], f32, kind="ExternalOutput")
    c_fin = nc.dram_tensor("c_fin", [H, bp], f32, kind="ExternalOutput")

    with tile.TileContext(nc) as tc:
        with (
            tc.tile_pool(name="consts", bufs=1) as cpool,
            tc.tile_pool(name="xt", bufs=2) as xtpool,
            tc.tile_pool(name="xproj", bufs=2) as xppool,
            tc.tile_pool(name="outw", bufs=2) as outpool,
            tc.tile_pool(name="state", bufs=1) as spool,
            tc.tile_pool(name="hmm", bufs=3) as hpool,
            tc.tile_pool(name="chain", bufs=3) as chpool,
            tc.tile_pool(name="pp", bufs=2, space="PSUM") as pppool,
            tc.tile_pool(name="gp", bufs=2, space="PSUM") as gppool,
            tc.tile_pool(name="gp1", bufs=1, space="PSUM") as gp1pool,
        ):
            whh_sb, wih_sb = [], []
            for k in range(NK):
                t = cpool.tile([128, G4], dtm, tag=f"whh{k}")
                nc.sync.dma_start(out=t[:, :], in_=whh[k * 128:(k + 1) * 128, :])
                whh_sb.append(t)
            for k in range(NK):
                t = cpool.tile([128, G4], dtm, tag=f"wih{k}")
                nc.sync.dma_start(out=t[:, :], in_=wih[k * 128:(k + 1) * 128, :])
                wih_sb.append(t)
            bias_sb = cpool.tile([128, NM], f32, tag="bias")
            nc.sync.dma_start(out=bias_sb[:, :], in_=bias[:, :])
            id_sb = cpool.tile([128, 128], dtm, tag="ident")
            nc.sync.dma_start(out=id_sb[:, :], in_=ident[:, :])

            # TC = [tanh_g | c] adjacent so fc/ig fuse into one DVE mul
            TC = spool.tile([128, 2 * HC], f32, tag="TC")
            for k in range(NK):
                nc.sync.dma_start(out=TC[:, HC + k * bp:HC + (k + 1) * bp],
                                  in_=c0[k * 128:(k + 1) * 128, :])
            hA = hpool.tile([128, 2 * bp], dtm, tag="hA")
            hB = hpool.tile([128, 2 * bp], dtm, tag="hB")
            for k in range(NK):
                dst = hA if k < 2 else hB
                nc.sync.dma_start(out=dst[:, (k % 2) * bp:(k % 2 + 1) * bp],
                                  in_=h0[k * 128:(k + 1) * 128, :])

            def emit_xt_dma(w):
                t = xtpool.tile([128, NK * W * bp], dtm, tag="xt")
                for k in range(NK):
                    nc.sync.dma_start(
                        out=t[:, k * W * bp:(k + 1) * W * bp],
                        in_=xt[k * 128:(k + 1) * 128, w * W * bp:(w + 1) * W * bp])
                return t

            def emit_proj_mm(t_xt, m):
                pp = pppool.tile([128, W * bp], f32, tag="pp")
                for k in range(NK):
                    nc.tensor.matmul(
                        pp[:, :],
                        lhsT=wih_sb[k][:, m * 128:(m + 1) * 128],
                        rhs=t_xt[:, k * W * bp:(k + 1) * W * bp],
                        start=(k == 0), stop=(k == NK - 1))
                return pp

            def emit_evac(xpw, pp, m):
                p = _pos_of_m(m)
                xpw3 = xpw[:, :].rearrange("p (w c) -> p w c", c=GCOLS)
                dst = xpw3[:, :, p * bp:(p + 1) * bp]
                pp3 = pp[:, :].rearrange("p (w c) -> p w c", c=bp)
                return nc.vector.tensor_scalar_add(dst, pp3,
                                                   bias_sb[:, m:m + 1])

            def emit_gates(s, xpw, h_prev):
                hA, hB = h_prev
                gp_g = gp1pool.tile([128, HC], f32, tag="gpg")
                gp_o = gppool.tile([128, HC], f32, tag="gpo")
                gp_i = gppool.tile([128, HC], f32, tag="gpi")
                gp_f = gp1pool.tile([128, HC], f32, tag="gpf")
                col0 = s * GCOLS
                for tgt, lo in ((gp_g, 0), (gp_o, HC), (gp_i, 2 * HC),
                                (gp_f, 3 * HC)):
                    nc.tensor.matmul(tgt[:, :], lhsT=id_sb[:, :],
                                     rhs=xpw[:, col0 + lo:col0 + lo + HC],
                                     start=True, stop=False,
                                     skip_group_check=True)
                si = chpool.tile([128, HC], f32, tag="si")
                so = chpool.tile([128, HC], f32, tag="so")
                sf = chpool.tile([128, HC], f32, tag="sf")
                P = chpool.tile([128, 2 * HC], f32, tag="P")
                tiles = {0: gp_g, 1: gp_o, 2: gp_i, 3: gp_f}
                for mi, m in enumerate(_M_ORDER):
                    p = _pos_of_m(m)
                    tgt, pc = tiles[p // 4], p % 4
                    for k in range(NK):
                        hk = hA if k < 2 else hB
                        nc.tensor.matmul(
                            tgt[:, pc * bp:(pc + 1) * bp],
                            lhsT=whh_sb[k][:, m * 128:(m + 1) * 128],
                            rhs=hk[:, (k % 2) * bp:(k % 2 + 1) * bp],
                            start=False,
                            stop=(mi == NM - 1 and k == NK - 1),
                            skip_group_check=True)
                    if mi == 3:     # f-groups done: sig_f + f*c hide here
                        nc.scalar.activation(sf[:, :], gp_f[:, :], Sig)
                        nc.vector.tensor_mul(P[:, HC:2 * HC], sf[:, :],
                                             TC[:, HC:2 * HC])
                    elif mi == 7:   # g-groups done
                        nc.scalar.activation(TC[:, 0:HC], gp_g[:, :], Tanh)
                    elif mi == 11:  # i-groups done
                        nc.scalar.activation(si[:, :], gp_i[:, :], Sig)
                        nc.vector.tensor_mul(P[:, 0:HC], si[:, :],
                                             TC[:, 0:HC])
                return gp_o, so, P

            def emit_chain(gp_o, so, P, h_new, hout_dst):
                hA, hB = h_new
                nc.scalar.activation(so[:, :], gp_o[:, :], Sig)
                nc.vector.tensor_add(TC[:, HC:2 * HC], P[:, 0:HC],
                                     P[:, HC:2 * HC])
                tcn = chpool.tile([128, HC], f32, tag="tcn")
                nc.scalar.activation(tcn[:, :], TC[:, HC:2 * HC], Tanh)
                mul1 = nc.vector.tensor_mul(hA[:, :], so[:, 0:2 * bp],
                                            tcn[:, 0:2 * bp])
                mul2 = nc.vector.tensor_mul(hB[:, :], so[:, 2 * bp:HC],
                                            tcn[:, 2 * bp:])
                nc.gpsimd.tensor_mul(hout_dst, so[:, 0:HC], tcn[:, :])
                return mul2

            # prologue: window 0 projection
            t_xt = emit_xt_dma(0)
            xpw = xppool.tile([128, W * GCOLS], dtm, tag="xpw")
            for m in range(NM):
                pp0 = emit_proj_mm(t_xt, m)
                emit_evac(xpw, pp0, m)

            h_prev = (hA, hB)
            outw = None
            prev_tail = None      # last chain DVE op of previous step
            pending_evac = None   # (pp, m) awaiting evacuation next step
            for w in range(NWIN):
                xpw_next = None
                if w + 1 < NWIN:
                    t_xt_next = emit_xt_dma(w + 1)
                    xpw_next = xppool.tile([128, W * GCOLS], dtm, tag="xpw")
                outw = outpool.tile([128, W * HC], f32, tag="outw")
                for s in range(W):
                    gp_o, so, P = emit_gates(s, xpw, h_prev)
                    if pending_evac is not None:
                        pp_p, m_p, xpw_p = pending_evac
                        with tc.high_priority(offset=-1000000):
                            ev = emit_evac(xpw_p, pp_p, m_p)
                        if prev_tail is not None:
                            add_dep_helper(ev.ins, prev_tail.ins, sync=False,
                                           reason="evac after prior chain")
                        pending_evac = None
                    if xpw_next is not None and s % 2 == 0:
                        with tc.high_priority(offset=-1000000):
                            pp = emit_proj_mm(t_xt_next, s // 2)
                        pending_evac = (pp, s // 2, xpw_next)
                    hA_n = hpool.tile([128, 2 * bp], dtm, tag="hA", name="hA_n")
                    hB_n = hpool.tile([128, 2 * bp], dtm, tag="hB", name="hB_n")
                    h_new = (hA_n, hB_n)
                    prev_tail = emit_chain(gp_o, so, P, h_new,
                                           outw[:, s * HC:(s + 1) * HC])
                    h_prev = h_new
                ow3 = outw[:, :].rearrange("p (w c) -> p w c", c=HC)
                for k in range(NK):
                    nc.sync.dma_start(
                        out=out_h[k * 128:(k + 1) * 128,
                                  w * W * bp:(w + 1) * W * bp],
                        in_=ow3[:, :, k * bp:(k + 1) * bp])
                xpw = xpw_next

            for k in range(NK):
                nc.sync.dma_start(
                    out=h_fin[k * 128:(k + 1) * 128, :],
                    in_=outw[:, (W - 1) * HC + k * bp:(W - 1) * HC + (k + 1) * bp])
                nc.sync.dma_start(out=c_fin[k * 128:(k + 1) * 128, :],
                                  in_=TC[:, HC + k * bp:HC + (k + 1) * bp])

    nc.finalize()
    return nc


_NC_CACHE = {}


def _get_nc(use_bf16):
    if use_bf16 not in _NC_CACHE:
        _NC_CACHE[use_bf16] = build_nc(use_bf16)
    return _NC_CACHE[use_bf16]


def _mm_np_dtype(use_bf16):
    return ml_dtypes.bfloat16 if use_bf16 else np.float32


def kernel(x, h0_f, c0_f, h0_b, c0_b, w_ih_f, w_hh_f, b_ih_f, b_hh_f,
           w_ih_b, w_hh_b, b_ih_b, b_hh_b):
    from concourse.bass_utils import run_bass_kernel_spmd

    use_bf16 = USE_BF16
    dt = _mm_np_dtype(use_bf16)
    x = np.asarray(x, np.float32)
    fwd = (np.asarray(h0_f, np.float32), np.asarray(c0_f, np.float32),
           np.asarray(w_ih_f, np.float32), np.asarray(w_hh_f, np.float32),
           np.asarray(b_ih_f, np.float32), np.asarray(b_hh_f, np.float32))
    bwd = (np.asarray(h0_b, np.float32), np.asarray(c0_b, np.float32),
           np.asarray(w_ih_b, np.float32), np.asarray(w_hh_b, np.float32),
           np.asarray(b_ih_b, np.float32), np.asarray(b_hh_b, np.float32))

    nc = _get_nc(use_bf16)
    ident = np.eye(128, dtype=dt)
    in_maps = []
    for core in range(NCORES):
        d, si = divmod(core, 4)
        sl = slice(si * bp, (si + 1) * bp)
        h0_, c0_, wih_, whh_, bih_, bhh_ = fwd if d == 0 else bwd
        xd = x if d == 0 else np.ascontiguousarray(x[::-1])
        xslc = np.ascontiguousarray(
            xd[:, sl, :].transpose(2, 0, 1)).reshape(I, T * bp)
        in_maps.append({
            "xt": xslc.astype(dt),
            "wih": np.ascontiguousarray(wih_.T).astype(dt),
            "whh": np.ascontiguousarray(whh_.T).astype(dt),
            "bias": np.ascontiguousarray(
                (bih_ + bhh_).reshape(NM, 128).T).astype(np.float32),
            "ident": ident,
            "h0": np.ascontiguousarray(h0_[sl].T).astype(dt),
            "c0": np.ascontiguousarray(c0_[sl].T).astype(np.float32),
        })

    trace = os.environ.get("LSTM_TRACE", "0") == "1"
    kwargs = {}
    if trace:
        kwargs["trace"] = True
        td = os.environ.get("LSTM_TRACE_DIR")
        if td:
            kwargs["tmpdir"] = td
    res = run_bass_kernel_spmd(nc, in_maps, core_ids=list(range(NCORES)),
                               **kwargs)
    kernel._last_results = res

    outputs = np.empty((T, B, 2 * H), np.float32)
    hf = np.empty((B, H), np.float32)
    cf = np.empty((B, H), np.float32)
    hb = np.empty((B, H), np.float32)
    cb = np.empty((B, H), np.float32)
    for core in range(NCORES):
        d, si = divmod(core, 4)
        sl = slice(si * bp, (si + 1) * bp)
        r = res.results[core]
        seq = r["out_h"].reshape(H, T, bp).transpose(1, 2, 0)
        if d == 1:
            seq = seq[::-1]
        outputs[:, sl, d * H:(d + 1) * H] = seq
        (hf if d == 0 else hb)[sl] = r["h_fin"].T
        (cf if d == 0 else cb)[sl] = r["c_fin"].T
    return outputs, hf, cf, hb, cb


# revision 29
# speedup vs baseline: 1.1529x; 1.0160x over previous
"""Bidirectional LSTM layer on 8 trn2 NeuronCores.

Sharding: cores 0-3 forward direction x batch slices of 16, cores 4-7
backward direction (input time-reversed on host) x the same batch slices.
Each core runs the full T=512 recurrence for its 16-batch slice in bf16
(fp32 cell state), with the input projection computed as a windowed bulk
GEMM (32 steps at a time, moving operand N=512) that is interleaved into
the PE stream at low priority so it fills the recurrence's dependency
bubbles.

On-chip layout is fully transposed: h, c stored [H, b] (H on partitions),
gates computed as gates^T = w^T-chunk . h into four per-gate PSUM tiles
(separate tiles so each gate's activation can start as soon as its own
matmuls finish - Tile dependency tracking is tile-granular). Per step the
x-projection (with bias prefolded) is injected into the gate PSUM banks
via an identity matmul with start=True, and the 64 recurrent matmuls
accumulate on top. Gate group order [f, g, i, o] hides sig(f), f*c,
tanh(g), sig(i) and i*tanh(g) under the gate matmul burst; only sig(o),
c_new, tanh(c) and the h-mul trail it.
"""

import os

import numpy as np
import ml_dtypes

T, B, I, H = 512, 64, 512, 512
G4 = 4 * H            # 2048 gate rows
NCORES = 8
bp = B // 4           # 16 batch per core
W = 32                # steps per window
NWIN = T // W         # 16
NM = G4 // 128        # 16 m-chunks
NK = H // 128         # 4 k-chunks
GCOLS = NM * bp       # 256 gate cols per step
HC = NK * bp          # 64 cols for h/c tiles

USE_BF16 = True      # matmul dtype switch

# column block position for m-chunk within the step's gate tile.
# order [g(0-3) | o(4-7) | i(8-11) | f(12-15)]
def _pos_of_m(m):
    j = m % 4
    if m < 4:
        return 8 + j      # i
    if m < 8:
        return 12 + j     # f
    if m < 12:
        return 0 + j      # g
    return 4 + j          # o


# PE emit order for gate m-chunks: f, g, i, o — f/g/i consumers hide
# under later gate matmuls; only sig_o + the c/h tail trail the burst.
_M_ORDER = [4, 5, 6, 7, 8, 9, 10, 11, 0, 1, 2, 3, 12, 13, 14, 15]


def build_nc(use_bf16):
    import concourse.bacc as bacc
    import concourse.mybir as mybir
    import concourse.tile as tile
    from concourse.tile_rust import add_dep_helper

    dtm = mybir.dt.bfloat16 if use_bf16 else mybir.dt.float32
    f32 = mybir.dt.float32
    Sig = mybir.ActivationFunctionType.Sigmoid
    Tanh = mybir.ActivationFunctionType.Tanh

    nc = bacc.Bacc("TRN2", target_bir_lowering=False, debug=False)

    xt = nc.dram_tensor("xt", [I, T * bp], dtm, kind="ExternalInput")
    wih = nc.dram_tensor("wih", [I, G4], dtm, kind="ExternalInput")
    whh = nc.dram_tensor("whh", [H, G4], dtm, kind="ExternalInput")
    bias = nc.dram_tensor("bias", [128, NM], f32, kind="ExternalInput")
    ident = nc.dram_tensor("ident", [128, 128], dtm, kind="ExternalInput")
    h0 = nc.dram_tensor("h0", [H, bp], dtm, kind="ExternalInput")
    c0 = nc.dram_tensor("c0", [H, bp], f32, kind="ExternalInput")
    out_h = nc.dram_tensor("out_h", [H, T * bp], f32, kind="ExternalOutput")
    h_fin = nc.dram_tensor("h_fin", [H, bp], f32, kind="ExternalOutput")
    c_fin = nc.dram_tensor("c_fin", [H, bp], f32, kind="ExternalOutput")

    with tile.TileContext(nc) as tc:
        with (
            tc.tile_pool(name="consts", bufs=1) as cpool,
            tc.tile_pool(name="xt", bufs=2) as xtpool,
            tc.tile_pool(name="xproj", bufs=2) as xppool,
            tc.tile_pool(name="outw", bufs=2) as outpool,
            tc.tile_pool(name="state", bufs=1) as spool,
            tc.tile_pool(name="hmm", bufs=3) as hpool,
            tc.tile_pool(name="chain", bufs=3) as chpool,
            tc.tile_pool(name="pp", bufs=2, space="PSUM") as pppool,
            tc.tile_pool(name="gp", bufs=2, space="PSUM") as gppool,
            tc.tile_pool(name="gp1", bufs=1, space="PSUM") as gp1pool,
        ):
            whh_sb, wih_sb = [], []
            for k in range(NK):
                t = cpool.tile([128, G4], dtm, tag=f"whh{k}")
                nc.sync.dma_start(out=t[:, :], in_=whh[k * 128:(k + 1) * 128, :])
                whh_sb.append(t)
            for k in range(NK):
                t = cpool.tile([128, G4], dtm, tag=f"wih{k}")
                nc.sync.dma_start(out=t[:, :], in_=wih[k * 128:(k + 1) * 128, :])
                wih_sb.append(t)
            bias_sb = cpool.tile([128, NM], f32, tag="bias")
            nc.sync.dma_start(out=bias_sb[:, :], in_=bias[:, :])
            id_sb = cpool.tile([128, 128], dtm, tag="ident")
            nc.sync.dma_start(out=id_sb[:, :], in_=ident[:, :])

            # TC = [tanh_g | c] adjacent so fc/ig fuse into one DVE mul
            TC = spool.tile([128, 2 * HC], f32, tag="TC")
            for k in range(NK):
                nc.sync.dma_start(out=TC[:, HC + k * bp:HC + (k + 1) * bp],
                                  in_=c0[k * 128:(k + 1) * 128, :])
            h_prev0 = hpool.tile([128, HC], dtm, tag="hmm")
            for k in range(NK):
                nc.sync.dma_start(
                    out=h_prev0[:, k * bp:(k + 1) * bp],
                    in_=h0[k * 128:(k + 1) * 128, :])

            def emit_xt_dma(w):
                t = xtpool.tile([128, NK * W * bp], dtm, tag="xt")
                for k in range(NK):
                    nc.sync.dma_start(
                        out=t[:, k * W * bp:(k + 1) * W * bp],
                        in_=xt[k * 128:(k + 1) * 128, w * W * bp:(w + 1) * W * bp])
                return t

            def emit_proj_mm(t_xt, m):
                pp = pppool.tile([128, W * bp], f32, tag="pp")
                for k in range(NK):
                    nc.tensor.matmul(
                        pp[:, :],
                        lhsT=wih_sb[k][:, m * 128:(m + 1) * 128],
                        rhs=t_xt[:, k * W * bp:(k + 1) * W * bp],
                        start=(k == 0), stop=(k == NK - 1))
                return pp

            def emit_evac(xpw, pp, m):
                p = _pos_of_m(m)
                xpw3 = xpw[:, :].rearrange("p (w c) -> p w c", c=GCOLS)
                dst = xpw3[:, :, p * bp:(p + 1) * bp]
                pp3 = pp[:, :].rearrange("p (w c) -> p w c", c=bp)
                return nc.vector.tensor_scalar_add(dst, pp3,
                                                   bias_sb[:, m:m + 1])

            def emit_gates(s, xpw, h_prev):
                gp_g = gp1pool.tile([128, HC], f32, tag="gpg")
                gp_o = gppool.tile([128, HC], f32, tag="gpo")
                gp_i = gppool.tile([128, HC], f32, tag="gpi")
                gp_f = gp1pool.tile([128, HC], f32, tag="gpf")
                col0 = s * GCOLS
                for tgt, lo in ((gp_g, 0), (gp_o, HC), (gp_i, 2 * HC),
                                (gp_f, 3 * HC)):
                    nc.tensor.matmul(tgt[:, :], lhsT=id_sb[:, :],
                                     rhs=xpw[:, col0 + lo:col0 + lo + HC],
                                     start=True, stop=False,
                                     skip_group_check=True)
                si = chpool.tile([128, HC], f32, tag="si")
                so = chpool.tile([128, HC], f32, tag="so")
                sf = chpool.tile([128, HC], f32, tag="sf")
                P = chpool.tile([128, 2 * HC], f32, tag="P")
                tiles = {0: gp_g, 1: gp_o, 2: gp_i, 3: gp_f}
                for mi, m in enumerate(_M_ORDER):
                    p = _pos_of_m(m)
                    tgt, pc = tiles[p // 4], p % 4
                    for k in range(NK):
                        nc.tensor.matmul(
                            tgt[:, pc * bp:(pc + 1) * bp],
                            lhsT=whh_sb[k][:, m * 128:(m + 1) * 128],
                            rhs=h_prev[:, k * bp:(k + 1) * bp],
                            start=False,
                            stop=(mi == NM - 1 and k == NK - 1),
                            skip_group_check=True)
                    if mi == 3:     # f-groups done: sig_f + f*c hide here
                        nc.scalar.activation(sf[:, :], gp_f[:, :], Sig)
                        nc.vector.tensor_mul(P[:, HC:2 * HC], sf[:, :],
                                             TC[:, HC:2 * HC])
                    elif mi == 7:   # g-groups done
                        nc.scalar.activation(TC[:, 0:HC], gp_g[:, :], Tanh)
                    elif mi == 11:  # i-groups done
                        nc.scalar.activation(si[:, :], gp_i[:, :], Sig)
                        nc.vector.tensor_mul(P[:, 0:HC], si[:, :],
                                             TC[:, 0:HC])
                return gp_o, so, P

            def emit_chain(gp_o, so, P, h_new, hout_dst):
                nc.scalar.activation(so[:, :], gp_o[:, :], Sig)
                nc.vector.tensor_add(TC[:, HC:2 * HC], P[:, 0:HC],
                                     P[:, HC:2 * HC])
                tcn = chpool.tile([128, HC], f32, tag="tcn")
                nc.scalar.activation(tcn[:, :], TC[:, HC:2 * HC], Tanh)
                mul1 = nc.vector.tensor_mul(h_new[:, :], so[:, :],
                                            tcn[:, :])
                nc.gpsimd.tensor_mul(hout_dst, so[:, :], tcn[:, :])
                return mul1

            # prologue: window 0 projection
            t_xt = emit_xt_dma(0)
            xpw = xppool.tile([128, W * GCOLS], dtm, tag="xpw")
            for m in range(NM):
                pp0 = emit_proj_mm(t_xt, m)
                emit_evac(xpw, pp0, m)

            h_prev = h_prev0
            outw = None
            prev_tail = None      # last chain DVE op of previous step
            pending_evac = None   # (pp, m) awaiting evacuation next step
            for w in range(NWIN):
                xpw_next = None
                if w + 1 < NWIN:
                    t_xt_next = emit_xt_dma(w + 1)
                    xpw_next = xppool.tile([128, W * GCOLS], dtm, tag="xpw")
                outw = outpool.tile([128, W * HC], f32, tag="outw")
                for s in range(W):
                    gp_o, so, P = emit_gates(s, xpw, h_prev)
                    if pending_evac is not None:
                        pp_p, m_p, xpw_p = pending_evac
                        with tc.high_priority(offset=-1000000):
                            ev = emit_evac(xpw_p, pp_p, m_p)
                        if prev_tail is not None:
                            add_dep_helper(ev.ins, prev_tail.ins, sync=False,
                                           reason="evac after prior chain")
                        pending_evac = None
                    if xpw_next is not None and s % 2 == 0:
                        with tc.high_priority(offset=-1000000):
                            pp = emit_proj_mm(t_xt_next, s // 2)
                        pending_evac = (pp, s // 2, xpw_next)
                    h_new = hpool.tile([128, HC], dtm, tag="hmm",
                                       name="h_new")
                    prev_tail = emit_chain(gp_o, so, P, h_new,
                                           outw[:, s * HC:(s + 1) * HC])
                    h_prev = h_new
                ow3 = outw[:, :].rearrange("p (w c) -> p w c", c=HC)
                for k in range(NK):
                    nc.sync.dma_start(
                        out=out_h[k * 128:(k + 1) * 128,
                                  w * W * bp:(w + 1) * W * bp],
                        in_=ow3[:, :, k * bp:(k + 1) * bp])
                xpw = xpw_next

            for k in range(NK):
                nc.sync.dma_start(
                    out=h_fin[k * 128:(k + 1) * 128, :],
                    in_=outw[:, (W - 1) * HC + k * bp:(W - 1) * HC + (k + 1) * bp])
                nc.sync.dma_start(out=c_fin[k * 128:(k + 1) * 128, :],
                                  in_=TC[:, HC + k * bp:HC + (k + 1) * bp])

    nc.finalize()
    return nc


_NC_CACHE = {}


def _get_nc(use_bf16):
    if use_bf16 not in _NC_CACHE:
        _NC_CACHE[use_bf16] = build_nc(use_bf16)
    return _NC_CACHE[use_bf16]


def _mm_np_dtype(use_bf16):
    return ml_dtypes.bfloat16 if use_bf16 else np.float32


def kernel(x, h0_f, c0_f, h0_b, c0_b, w_ih_f, w_hh_f, b_ih_f, b_hh_f,
           w_ih_b, w_hh_b, b_ih_b, b_hh_b):
    from concourse.bass_utils import run_bass_kernel_spmd

    use_bf16 = USE_BF16
    dt = _mm_np_dtype(use_bf16)
    x = np.asarray(x, np.float32)
    fwd = (np.asarray(h0_f, np.float32), np.asarray(c0_f, np.float32),
           np.asarray(w_ih_f, np.float32), np.asarray(w_hh_f, np.float32),
           np.asarray(b_ih_f, np.float32), np.asarray(b_hh_f, np.float32))
    bwd = (np.asarray(h0_b, np.float32), np.asarray(c0_b, np.float32),
           np.asarray(w_ih_b, np.float32), np.asarray(w_hh_b, np.float32),
           np.asarray(b_ih_b, np.float32), np.asarray(b_hh_b, np.float32))

    nc = _get_nc(use_bf16)
    ident = np.eye(128, dtype=dt)
    in_maps = []
    for core in range(NCORES):
        d, si = divmod(core, 4)
        sl = slice(si * bp, (si + 1) * bp)
        h0_, c0_, wih_, whh_, bih_, bhh_ = fwd if d == 0 else bwd
        xd = x if d == 0 else np.ascontiguousarray(x[::-1])
        xslc = np.ascontiguousarray(
            xd[:, sl, :].transpose(2, 0, 1)).reshape(I, T * bp)
        in_maps.append({
            "xt": xslc.astype(dt),
            "wih": np.ascontiguousarray(wih_.T).astype(dt),
            "whh": np.ascontiguousarray(whh_.T).astype(dt),
            "bias": np.ascontiguousarray(
                (bih_ + bhh_).reshape(NM, 128).T).astype(np.float32),
            "ident": ident,
            "h0": np.ascontiguousarray(h0_[sl].T).astype(dt),
            "c0": np.ascontiguousarray(c0_[sl].T).astype(np.float32),
        })

    trace = os.environ.get("LSTM_TRACE", "0") == "1"
    kwargs = {}
    if trace:
        kwargs["trace"] = True
        td = os.environ.get("LSTM_TRACE_DIR")
        if td:
            kwargs["tmpdir"] = td
    res = run_bass_kernel_spmd(nc, in_maps, core_ids=list(range(NCORES)),
                               **kwargs)
    kernel._last_results = res

    outputs = np.empty((T, B, 2 * H), np.float32)
    hf = np.empty((B, H), np.float32)
    cf = np.empty((B, H), np.float32)
    hb = np.empty((B, H), np.float32)
    cb = np.empty((B, H), np.float32)
    for core in range(NCORES):
        d, si = divmod(core, 4)
        sl = slice(si * bp, (si + 1) * bp)
        r = res.results[core]
        seq = r["out_h"].reshape(H, T, bp).transpose(1, 2, 0)
        if d == 1:
            seq = seq[::-1]
        outputs[:, sl, d * H:(d + 1) * H] = seq
        (hf if d == 0 else hb)[sl] = r["h_fin"].T
        (cf if d == 0 else cb)[sl] = r["c_fin"].T
    return outputs, hf, cf, hb, cb


# revision 30
# speedup vs baseline: 1.1564x; 1.0030x over previous
"""Bidirectional LSTM layer on 8 trn2 NeuronCores.

Sharding: cores 0-3 forward direction x batch slices of 16, cores 4-7
backward direction (input time-reversed on host) x the same batch slices.
Each core runs the full T=512 recurrence for its 16-batch slice in bf16
(fp32 cell state), with the input projection computed as a windowed bulk
GEMM (32 steps at a time, moving operand N=512) that is interleaved into
the PE stream at low priority so it fills the recurrence's dependency
bubbles.

On-chip layout is fully transposed: h, c stored [H, b] (H on partitions),
gates computed as gates^T = w^T-chunk . h into four per-gate PSUM tiles
(separate tiles so each gate's activation can start as soon as its own
matmuls finish - Tile dependency tracking is tile-granular). Per step the
x-projection (with bias prefolded) is injected into the gate PSUM banks
via an identity matmul with start=True, and the 64 recurrent matmuls
accumulate on top. Gate group order [f, g, i, o] hides sig(f), f*c,
tanh(g), sig(i) and i*tanh(g) under the gate matmul burst; only sig(o),
c_new, tanh(c) and the h-mul trail it.
"""

import os

import numpy as np
import ml_dtypes

T, B, I, H = 512, 64, 512, 512
G4 = 4 * H            # 2048 gate rows
NCORES = 8
bp = B // 4           # 16 batch per core
W = 32                # steps per window
NWIN = T // W         # 16
NM = G4 // 128        # 16 m-chunks
NK = H // 128         # 4 k-chunks
GCOLS = NM * bp       # 256 gate cols per step
HC = NK * bp          # 64 cols for h/c tiles

USE_BF16 = True      # matmul dtype switch

# column block position for m-chunk within the step's gate tile.
# order [g(0-3) | o(4-7) | i(8-11) | f(12-15)]
def _pos_of_m(m):
    j = m % 4
    if m < 4:
        return 8 + j      # i
    if m < 8:
        return 12 + j     # f
    if m < 12:
        return 0 + j      # g
    return 4 + j          # o


# PE emit order for gate m-chunks: f, g, i, o — f/g/i consumers hide
# under later gate matmuls; only sig_o + the c/h tail trail the burst.
_M_ORDER = [4, 5, 6, 7, 8, 9, 10, 11, 0, 1, 2, 3, 12, 13, 14, 15]


def build_nc(use_bf16):
    import concourse.bacc as bacc
    import concourse.mybir as mybir
    import concourse.tile as tile
    from concourse.tile_rust import add_dep_helper

    dtm = mybir.dt.bfloat16 if use_bf16 else mybir.dt.float32
    f32 = mybir.dt.float32
    Sig = mybir.ActivationFunctionType.Sigmoid
    Tanh = mybir.ActivationFunctionType.Tanh

    nc = bacc.Bacc("TRN2", target_bir_lowering=False, debug=False)

    xt = nc.dram_tensor("xt", [I, T * bp], dtm, kind="ExternalInput")
    wih = nc.dram_tensor("wih", [I, G4], dtm, kind="ExternalInput")
    whh = nc.dram_tensor("whh", [H, G4], dtm, kind="ExternalInput")
    bias = nc.dram_tensor("bias", [128, NM], f32, kind="ExternalInput")
    ident = nc.dram_tensor("ident", [128, 128], dtm, kind="ExternalInput")
    h0 = nc.dram_tensor("h0", [H, bp], dtm, kind="ExternalInput")
    c0 = nc.dram_tensor("c0", [H, bp], f32, kind="ExternalInput")
    out_h = nc.dram_tensor("out_h", [H, T * bp], f32, kind="ExternalOutput")
    h_fin = nc.dram_tensor("h_fin", [H, bp], f32, kind="ExternalOutput")
    c_fin = nc.dram_tensor("c_fin", [H, bp], f32, kind="ExternalOutput")

    with tile.TileContext(nc) as tc:
        with (
            tc.tile_pool(name="consts", bufs=1) as cpool,
            tc.tile_pool(name="xt", bufs=2) as xtpool,
            tc.tile_pool(name="xproj", bufs=2) as xppool,
            tc.tile_pool(name="outw", bufs=2) as outpool,
            tc.tile_pool(name="state", bufs=1) as spool,
            tc.tile_pool(name="hmm", bufs=3) as hpool,
            tc.tile_pool(name="chain", bufs=3) as chpool,
            tc.tile_pool(name="pp", bufs=2, space="PSUM") as pppool,
            tc.tile_pool(name="gp", bufs=2, space="PSUM") as gppool,
            tc.tile_pool(name="gp1", bufs=1, space="PSUM") as gp1pool,
        ):
            whh_sb, wih_sb = [], []
            for k in range(NK):
                t = cpool.tile([128, G4], dtm, tag=f"whh{k}")
                nc.sync.dma_start(out=t[:, :], in_=whh[k * 128:(k + 1) * 128, :])
                whh_sb.append(t)
            for k in range(NK):
                t = cpool.tile([128, G4], dtm, tag=f"wih{k}")
                nc.sync.dma_start(out=t[:, :], in_=wih[k * 128:(k + 1) * 128, :])
                wih_sb.append(t)
            bias_sb = cpool.tile([128, NM], f32, tag="bias")
            nc.sync.dma_start(out=bias_sb[:, :], in_=bias[:, :])
            id_sb = cpool.tile([128, 128], dtm, tag="ident")
            nc.sync.dma_start(out=id_sb[:, :], in_=ident[:, :])

            # TC = [tanh_g | c] adjacent so fc/ig fuse into one DVE mul
            TC = spool.tile([128, 2 * HC], f32, tag="TC")
            for k in range(NK):
                nc.sync.dma_start(out=TC[:, HC + k * bp:HC + (k + 1) * bp],
                                  in_=c0[k * 128:(k + 1) * 128, :])
            h_prev0 = hpool.tile([128, HC], dtm, tag="hmm")
            for k in range(NK):
                nc.sync.dma_start(
                    out=h_prev0[:, k * bp:(k + 1) * bp],
                    in_=h0[k * 128:(k + 1) * 128, :])

            def emit_xt_dma(w):
                t = xtpool.tile([128, NK * W * bp], dtm, tag="xt")
                for k in range(NK):
                    nc.sync.dma_start(
                        out=t[:, k * W * bp:(k + 1) * W * bp],
                        in_=xt[k * 128:(k + 1) * 128, w * W * bp:(w + 1) * W * bp])
                return t

            def emit_proj_mm(t_xt, m):
                pp = pppool.tile([128, W * bp], f32, tag="pp")
                for k in range(NK):
                    nc.tensor.matmul(
                        pp[:, :],
                        lhsT=wih_sb[k][:, m * 128:(m + 1) * 128],
                        rhs=t_xt[:, k * W * bp:(k + 1) * W * bp],
                        start=(k == 0), stop=(k == NK - 1))
                return pp

            def emit_evac(xpw, pp, m):
                p = _pos_of_m(m)
                xpw3 = xpw[:, :].rearrange("p (w c) -> p w c", c=GCOLS)
                dst = xpw3[:, :, p * bp:(p + 1) * bp]
                pp3 = pp[:, :].rearrange("p (w c) -> p w c", c=bp)
                return nc.vector.tensor_scalar_add(dst, pp3,
                                                   bias_sb[:, m:m + 1])

            def emit_gates(s, xpw, h_prev):
                gp_g = gp1pool.tile([128, HC], f32, tag="gpg")
                gp_o = gppool.tile([128, HC], f32, tag="gpo")
                gp_i = gppool.tile([128, HC], f32, tag="gpi")
                gp_f = gp1pool.tile([128, HC], f32, tag="gpf")
                col0 = s * GCOLS
                for tgt, lo in ((gp_g, 0), (gp_o, HC), (gp_i, 2 * HC),
                                (gp_f, 3 * HC)):
                    nc.tensor.matmul(tgt[:, :], lhsT=id_sb[:, :],
                                     rhs=xpw[:, col0 + lo:col0 + lo + HC],
                                     start=True, stop=False,
                                     skip_group_check=True)
                si = chpool.tile([128, HC], f32, tag="si")
                so = chpool.tile([128, HC], f32, tag="so")
                sf = chpool.tile([128, HC], f32, tag="sf")
                P = chpool.tile([128, 2 * HC], f32, tag="P")
                tiles = {0: gp_g, 1: gp_o, 2: gp_i, 3: gp_f}
                last_o_mm = None
                for mi, m in enumerate(_M_ORDER):
                    p = _pos_of_m(m)
                    tgt, pc = tiles[p // 4], p % 4
                    for k in range(NK):
                        mm = nc.tensor.matmul(
                            tgt[:, pc * bp:(pc + 1) * bp],
                            lhsT=whh_sb[k][:, m * 128:(m + 1) * 128],
                            rhs=h_prev[:, k * bp:(k + 1) * bp],
                            start=False,
                            stop=(mi == NM - 1 and k == NK - 1),
                            skip_group_check=True)
                    # dedicated sync edges: release each gate's activation
                    # right when its own matmuls finish, instead of at the
                    # PE's next (batched) semaphore update point.
                    if mi == 3:     # f-groups done: sig_f + f*c hide here
                        act = nc.scalar.activation(sf[:, :], gp_f[:, :], Sig)
                        add_dep_helper(act.ins, mm.ins, sync=True,
                                       reason="sig_f right after f MMs")
                        nc.vector.tensor_mul(P[:, HC:2 * HC], sf[:, :],
                                             TC[:, HC:2 * HC])
                    elif mi == 7:   # g-groups done
                        act = nc.scalar.activation(TC[:, 0:HC], gp_g[:, :],
                                                   Tanh)
                        add_dep_helper(act.ins, mm.ins, sync=True,
                                       reason="tanh_g right after g MMs")
                    elif mi == 11:  # i-groups done
                        act = nc.scalar.activation(si[:, :], gp_i[:, :], Sig)
                        add_dep_helper(act.ins, mm.ins, sync=True,
                                       reason="sig_i right after i MMs")
                        nc.vector.tensor_mul(P[:, 0:HC], si[:, :],
                                             TC[:, 0:HC])
                    elif mi == 15:
                        last_o_mm = mm
                return gp_o, so, P, last_o_mm

            def emit_chain(gp_o, so, P, h_new, hout_dst, last_o_mm):
                act_o = nc.scalar.activation(so[:, :], gp_o[:, :], Sig)
                add_dep_helper(act_o.ins, last_o_mm.ins, sync=True,
                               reason="sig_o right after o MMs")
                nc.vector.tensor_add(TC[:, HC:2 * HC], P[:, 0:HC],
                                     P[:, HC:2 * HC])
                tcn = chpool.tile([128, HC], f32, tag="tcn")
                nc.scalar.activation(tcn[:, :], TC[:, HC:2 * HC], Tanh)
                mul1 = nc.vector.tensor_mul(h_new[:, :], so[:, :],
                                            tcn[:, :])
                nc.gpsimd.tensor_mul(hout_dst, so[:, :], tcn[:, :])
                return mul1

            # prologue: window 0 projection
            t_xt = emit_xt_dma(0)
            xpw = xppool.tile([128, W * GCOLS], dtm, tag="xpw")
            for m in range(NM):
                pp0 = emit_proj_mm(t_xt, m)
                emit_evac(xpw, pp0, m)

            h_prev = h_prev0
            outw = None
            prev_tail = None      # last chain DVE op of previous step
            pending_evac = None   # (pp, m) awaiting evacuation next step
            for w in range(NWIN):
                xpw_next = None
                if w + 1 < NWIN:
                    t_xt_next = emit_xt_dma(w + 1)
                    xpw_next = xppool.tile([128, W * GCOLS], dtm, tag="xpw")
                outw = outpool.tile([128, W * HC], f32, tag="outw")
                for s in range(W):
                    gp_o, so, P, last_o_mm = emit_gates(s, xpw, h_prev)
                    if pending_evac is not None:
                        pp_p, m_p, xpw_p = pending_evac
                        with tc.high_priority(offset=-1000000):
                            ev = emit_evac(xpw_p, pp_p, m_p)
                        if prev_tail is not None:
                            add_dep_helper(ev.ins, prev_tail.ins, sync=False,
                                           reason="evac after prior chain")
                        pending_evac = None
                    if xpw_next is not None and s % 2 == 0:
                        with tc.high_priority(offset=-1000000):
                            pp = emit_proj_mm(t_xt_next, s // 2)
                        pending_evac = (pp, s // 2, xpw_next)
                    h_new = hpool.tile([128, HC], dtm, tag="hmm",
                                       name="h_new")
                    prev_tail = emit_chain(gp_o, so, P, h_new,
                                           outw[:, s * HC:(s + 1) * HC],
                                           last_o_mm)
                    h_prev = h_new
                ow3 = outw[:, :].rearrange("p (w c) -> p w c", c=HC)
                for k in range(NK):
                    nc.sync.dma_start(
                        out=out_h[k * 128:(k + 1) * 128,
                                  w * W * bp:(w + 1) * W * bp],
                        in_=ow3[:, :, k * bp:(k + 1) * bp])
                xpw = xpw_next

            for k in range(NK):
                nc.sync.dma_start(
                    out=h_fin[k * 128:(k + 1) * 128, :],
                    in_=outw[:, (W - 1) * HC + k * bp:(W - 1) * HC + (k + 1) * bp])
                nc.sync.dma_start(out=c_fin[k * 128:(k + 1) * 128, :],
                                  in_=TC[:, HC + k * bp:HC + (k + 1) * bp])

    nc.finalize()
    return nc


_NC_CACHE = {}


def _get_nc(use_bf16):
    if use_bf16 not in _NC_CACHE:
        _NC_CACHE[use_bf16] = build_nc(use_bf16)
    return _NC_CACHE[use_bf16]


def _mm_np_dtype(use_bf16):
    return ml_dtypes.bfloat16 if use_bf16 else np.float32


def kernel(x, h0_f, c0_f, h0_b, c0_b, w_ih_f, w_hh_f, b_ih_f, b_hh_f,
           w_ih_b, w_hh_b, b_ih_b, b_hh_b):
    from concourse.bass_utils import run_bass_kernel_spmd

    use_bf16 = USE_BF16
    dt = _mm_np_dtype(use_bf16)
    x = np.asarray(x, np.float32)
    fwd = (np.asarray(h0_f, np.float32), np.asarray(c0_f, np.float32),
           np.asarray(w_ih_f, np.float32), np.asarray(w_hh_f, np.float32),
           np.asarray(b_ih_f, np.float32), np.asarray(b_hh_f, np.float32))
    bwd = (np.asarray(h0_b, np.float32), np.asarray(c0_b, np.float32),
           np.asarray(w_ih_b, np.float32), np.asarray(w_hh_b, np.float32),
           np.asarray(b_ih_b, np.float32), np.asarray(b_hh_b, np.float32))

    nc = _get_nc(use_bf16)
    ident = np.eye(128, dtype=dt)
    in_maps = []
    for core in range(NCORES):
        d, si = divmod(core, 4)
        sl = slice(si * bp, (si + 1) * bp)
        h0_, c0_, wih_, whh_, bih_, bhh_ = fwd if d == 0 else bwd
        xd = x if d == 0 else np.ascontiguousarray(x[::-1])
        xslc = np.ascontiguousarray(
            xd[:, sl, :].transpose(2, 0, 1)).reshape(I, T * bp)
        in_maps.append({
            "xt": xslc.astype(dt),
            "wih": np.ascontiguousarray(wih_.T).astype(dt),
            "whh": np.ascontiguousarray(whh_.T).astype(dt),
            "bias": np.ascontiguousarray(
                (bih_ + bhh_).reshape(NM, 128).T).astype(np.float32),
            "ident": ident,
            "h0": np.ascontiguousarray(h0_[sl].T).astype(dt),
            "c0": np.ascontiguousarray(c0_[sl].T).astype(np.float32),
        })

    trace = os.environ.get("LSTM_TRACE", "0") == "1"
    kwargs = {}
    if trace:
        kwargs["trace"] = True
        td = os.environ.get("LSTM_TRACE_DIR")
        if td:
            kwargs["tmpdir"] = td
    res = run_bass_kernel_spmd(nc, in_maps, core_ids=list(range(NCORES)),
                               **kwargs)
    kernel._last_results = res

    outputs = np.empty((T, B, 2 * H), np.float32)
    hf = np.empty((B, H), np.float32)
    cf = np.empty((B, H), np.float32)
    hb = np.empty((B, H), np.float32)
    cb = np.empty((B, H), np.float32)
    for core in range(NCORES):
        d, si = divmod(core, 4)
        sl = slice(si * bp, (si + 1) * bp)
        r = res.results[core]
        seq = r["out_h"].reshape(H, T, bp).transpose(1, 2, 0)
        if d == 1:
            seq = seq[::-1]
        outputs[:, sl, d * H:(d + 1) * H] = seq
        (hf if d == 0 else hb)[sl] = r["h_fin"].T
        (cf if d == 0 else cb)[sl] = r["c_fin"].T
    return outputs, hf, cf, hb, cb
